# revision 1
# baseline (speedup 1.0000x reference)
"""CFConv (SchNet continuous-filter convolution) on 8 TRN2 NeuronCores, v2.

    h   = softplus(rbf @ w1 + b1)        # [N, NB, F]
    W   = h @ w2 + b2                    # [N, NB, F]
    out = sum_n x[neighbors] * W         # [N, F]

Sharding: atoms split 8 ways; x + filter weights replicated. No collectives.

v2 layout: per core, atoms padded to 2560 = 20 groups of 128. A span = one
group = 4096 pairs, pair index i = n*128 + a (neighbor-major within group).

Per-span dataflow:
  mm1 (PE):    ph[g, i] = w1[r, g].T @ rbf_t[r, i]        (feature-major)
  ACT:         es = exp(ph); hsp = ln(1 + es)  = softplus  (fp16)
  mm2 (PE):    pw[a, n*128+f] = hsp[:, n-block].T @ w2     (PAIR-major out:
               stationary = h block, moving = w2; out partition = atom)
  gather:      xj[a, n, f] = x[nbr] via NON-transpose dma_gather.  Pair i
               lands at partition i%128 = a, column i//128 = n -- exactly
               matching pw.  Gathers spread over SWDGE queues 0-3 (the Q7
               core pair is per-queue, so 4 gathers run concurrently;
               transpose-mode gathers CANNOT overlap -- shared XBAR sprays
               interleave and corrupt; non-transpose descriptors are
               self-contained, verified correct).
  DVE:         prod = pw * xj; then sum over n = 5 contiguous-half adds
               (n is the slow index, so every tree level is unit-stride).
  out:         r5[a, f] f32 -> DRAM rows [g*128, (g+1)*128)  (atom-major,
               no host transpose).

b1 rides the ones-row of w1 (RK=65).  b2 is zero in this problem; when
nonzero it is folded in with a rank-1 PE accumulate (ones ⊗ b2) per pw tile.
"""

import os

import numpy as np

import concourse.bass as bass
import concourse.bacc as bacc
import concourse.mybir as mybir
import concourse.tile as tile
from contextlib import ExitStack

N_ATOMS = 20000
NB = 32
F = 128
R = 64
RK = R + 1                      # mm1 contraction rows: 64 rbf dims + ones row (b1)
NCORES = 8
NA = N_ATOMS // NCORES          # real atoms per core       = 2500
GROUPS = 20                     # atom groups of 128 per core (padded)
NAP = GROUPS * 128              # padded atoms per core      = 2560
SPAN = 128 * NB                 # pairs per span (one group) = 4096
NPP = GROUPS * SPAN             # padded pairs per core      = 81920

f16 = mybir.dt.float16
f32 = mybir.dt.float32
i16 = mybir.dt.int16

_CACHE = {}


class _Bacc(bacc.Bacc):
    """Bacc with Exp+Ln pinned to the one activation table that holds both.

    The greedy table chooser otherwise alternates exp_and_others /
    natural_log every span (2 ACT_TABLE_LOADs x 1.3us each per span).
    Table ids (list positions) are unchanged -- we only stop advertising
    Exp/Ln in the other tables, which genuinely do contain them anyway.
    """

    def insert_act_table_loads(self):
        import bass_rust as _bass_rust
        from concourse.hw_specs import get_activation_tables

        both = {
            mybir.ActivationFunctionType.Exp,
            mybir.ActivationFunctionType.Ln,
        }
        tables = []
        for name, funcs in get_activation_tables(self.m.arch).items():
            if name != "natural_log_exp_and_others":
                funcs = funcs - both
            tables.append((name, funcs))
        _bass_rust.insert_act_table_loads(self, tables)


def _build(with_b2: bool):
    key = ("nc", with_b2)
    if key in _CACHE:
        return _CACHE[key]
    nc = _Bacc(num_swdge_queues=4)

    x_d = nc.declare_dram_parameter("x", [N_ATOMS, F], f16, isOutput=False)
    rbf_d = nc.declare_dram_parameter("rbf_t", [RK, NPP], f16, isOutput=False)
    idx_d = nc.declare_dram_parameter("idx", [128, NPP // 16], i16, isOutput=False)
    w1_d = nc.declare_dram_parameter("w1", [RK, F], f16, isOutput=False)
    w2_d = nc.declare_dram_parameter("w2", [F, F], f16, isOutput=False)
    out_d = nc.declare_dram_parameter("out", [NAP, F], f32, isOutput=True)
    if with_b2:
        b2_d = nc.declare_dram_parameter("b2rep", [1, 1024], f16, isOutput=False)

    with tile.TileContext(nc) as tc, ExitStack() as ctx:
        consts = ctx.enter_context(tc.tile_pool(name="consts", bufs=1))
        spool = ctx.enter_context(tc.tile_pool(name="spool", bufs=2))
        xpool = ctx.enter_context(tc.tile_pool(name="xpool", bufs=10))
        xqpool = ctx.enter_context(tc.tile_pool(name="xqpool", bufs=6))
        rpool = ctx.enter_context(tc.tile_pool(name="rpool", bufs=2))
        # rbf loads run well ahead of compute so the 512KB-per-span DMAs
        # never contend with the final gathers' payload drain.
        rbpool = ctx.enter_context(tc.tile_pool(name="rbpool", bufs=5))
        ph_pool = ctx.enter_context(tc.tile_pool(name="ph", bufs=2, space="PSUM"))
        pw_pool = ctx.enter_context(tc.tile_pool(name="pw", bufs=2, space="PSUM"))

        # Warmup gather (16 zero indices): the first dma_gather pays a ~6us
        # Q7 library IRAM load; issue a tiny one immediately so it overlaps
        # the idx upload and the first real gather starts hot.
        idxw = consts.tile([128, 1], i16)
        nc.vector.memset(idxw, 0)
        xw = consts.tile([128, F], f16)
        nc.gpsimd.dma_gather(
            xw.rearrange("p (c f) -> p c f", f=F),
            x_d[:],
            idxw[:],
            16,
            16,
            F,
            transpose=False,
            single_packet=False,
            queue_num=0,
        )
        # Span 0's indices land first so its gathers aren't gated on the
        # full 1.25MB idx upload.
        SP0C = SPAN // 16  # idx cols for one span
        idx0 = consts.tile([128, SP0C], i16)
        nc.sync.dma_start(out=idx0, in_=idx_d[:, :SP0C])
        w1s = consts.tile([RK, F], f16)
        nc.sync.dma_start(out=w1s, in_=w1_d[:])
        w2s = consts.tile([F, F], f16)
        nc.sync.dma_start(out=w2s, in_=w2_d[:])
        idxs = consts.tile([128, NPP // 16 - SP0C], i16)
        nc.sync.dma_start(out=idxs, in_=idx_d[:, SP0C:])
        if with_b2:
            b2s = consts.tile([1, 1024], f16)
            nc.sync.dma_start(out=b2s, in_=b2_d[:])
            ones1 = consts.tile([1, F], f16)
            nc.vector.memset(ones1, 1.0)

        gather_i = 0
        for g in range(GROUPS):
            s0 = g * SPAN

            rbft = rbpool.tile([RK, SPAN], f16, tag="rbft")
            nc.sync.dma_start(out=rbft, in_=rbf_d[:, s0 : s0 + SPAN])

            # Half-span gathers spread across the 4 SWDGE queues (each queue
            # owns a Q7 core pair, so 4 run concurrently; 2048-idx pieces
            # keep the descriptor rings from backing up).  The last two
            # spans use quarter-span pieces so their completion semaphores
            # fire incrementally -- otherwise the final DMA drain stalls
            # the whole tail ~25us.
            parts = 4
            pw_cols = SPAN // parts
            xjh = []
            for h in range(parts):
                pool = xpool if parts == 2 else xqpool
                xj = pool.tile([128, pw_cols], f16, tag=f"xj{h}_{parts}")
                i = gather_i
                gather_i += 1
                h0 = s0 + h * pw_cols
                if g == 0:
                    isrc = idx0[:, h0 // 16 : (h0 + pw_cols) // 16]
                else:
                    isrc = idxs[:, h0 // 16 - SP0C : (h0 + pw_cols) // 16 - SP0C]
                nc.gpsimd.dma_gather(
                    xj.rearrange("p (c f) -> p c f", f=F),
                    x_d[:],
                    isrc,
                    pw_cols,
                    pw_cols,
                    F,
                    transpose=False,
                    single_packet=False,
                    queue_num=(1, 2, 3, 0)[i % 4],
                )
                xjh.append(xj)

            # mm1 + exp in 1024-col chunks (ph = 2 PSUM banks); es/hsp are
            # half-span tiles so ln (and then mm2) can start after half the
            # exps instead of all four.
            es = [
                spool.tile([128, SPAN // 2], f16, tag=f"es{h}", name=f"es{h}_{g}")
                for h in range(2)
            ]
            hsp = [
                spool.tile([128, SPAN // 2], f16, tag=f"hsp{h}", name=f"hsp{h}_{g}")
                for h in range(2)
            ]
            for c in range(0, SPAN, 1024):
                ph = ph_pool.tile([128, 1024], f32)
                for o in range(0, 1024, 512):
                    nc.tensor.matmul(
                        ph[:, o : o + 512],
                        w1s[:],
                        rbft[:, c + o : c + o + 512],
                        start=True,
                        stop=True,
                    )
                nc.scalar.activation(
                    out=es[c // 2048][:, c % 2048 : c % 2048 + 1024],
                    in_=ph[:],
                    func=mybir.ActivationFunctionType.Exp,
                    bias=0.0,
                    scale=1.0,
                )
                if c % 2048 == 1024:  # softplus = ln(1 + e^x) per half-span
                    nc.scalar.activation(
                        out=hsp[c // 2048],
                        in_=es[c // 2048],
                        func=mybir.ActivationFunctionType.Ln,
                        bias=1.0,
                        scale=1.0,
                    )

            # mm2 pair-major + product, per 1024-col pw tile (= 8 n-blocks)
            prod = spool.tile([128, SPAN], f16, tag="prod")
            for t in range(SPAN // 1024):
                pw = pw_pool.tile([128, 1024], f32)
                for b in range(8):
                    n = t * 8 + b
                    nc.tensor.matmul(
                        pw[:, b * 128 : (b + 1) * 128],
                        hsp[n // 16][:, (n % 16) * 128 : (n % 16 + 1) * 128],
                        w2s[:],
                        start=True,
                        stop=not with_b2,
                    )
                if with_b2:
                    for o in range(0, 1024, 512):
                        nc.tensor.matmul(
                            pw[:, o : o + 512],
                            ones1[:],
                            b2s[:, o : o + 512],
                            start=False,
                            stop=True,
                        )
                if parts == 2:
                    xsrc = xjh[t // 2][:, (t % 2) * 1024 : (t % 2 + 1) * 1024]
                else:
                    xsrc = xjh[t][:]
                nc.vector.tensor_tensor(
                    out=prod[:, t * 1024 : (t + 1) * 1024],
                    in0=pw[:],
                    in1=xsrc,
                    op=mybir.AluOpType.mult,
                )

            # neighbor sum: n is the slow index -> contiguous-half tree
            r1 = rpool.tile([128, SPAN // 2], f16, tag="r1")
            nc.vector.tensor_tensor(
                out=r1, in0=prod[:, : SPAN // 2], in1=prod[:, SPAN // 2 :],
                op=mybir.AluOpType.add,
            )
            r2 = rpool.tile([128, SPAN // 4], f16, tag="r2")
            nc.vector.tensor_tensor(
                out=r2, in0=r1[:, : SPAN // 4], in1=r1[:, SPAN // 4 :],
                op=mybir.AluOpType.add,
            )
            r3 = rpool.tile([128, SPAN // 8], f16, tag="r3")
            nc.vector.tensor_tensor(
                out=r3, in0=r2[:, : SPAN // 8], in1=r2[:, SPAN // 8 :],
                op=mybir.AluOpType.add,
            )
            r4 = rpool.tile([128, SPAN // 16], f16, tag="r4")
            nc.vector.tensor_tensor(
                out=r4, in0=r3[:, : SPAN // 16], in1=r3[:, SPAN // 16 :],
                op=mybir.AluOpType.add,
            )
            r5 = rpool.tile([128, F], f32, tag="r5")
            nc.vector.tensor_tensor(
                out=r5, in0=r4[:, :F], in1=r4[:, F:],
                op=mybir.AluOpType.add,
            )
            nc.sync.dma_start(out=out_d[g * 128 : (g + 1) * 128, :], in_=r5)

    nc.finalize()
    _CACHE[key] = nc
    return nc


def _prep_core_inputs(x16, rbf, neighbors, w1a_16, w2_16, b2rep, c):
    a0 = c * NA
    # pad this core's 2500 atoms to 2560
    rbf_c = np.zeros((NAP, NB, R), dtype=np.float32)
    rbf_c[:NA] = rbf[a0 : a0 + NA]
    nb_c = np.zeros((NAP, NB), dtype=np.int64)
    nb_c[:NA] = neighbors[a0 : a0 + NA]

    # rbf_t[r, g*4096 + n*128 + a] = rbf_c[g*128 + a, n, r]
    rbf_t = np.empty((RK, NPP), dtype=np.float16)
    rbf_t[:R] = (
        rbf_c.reshape(GROUPS, 128, NB, R)
        .transpose(3, 0, 2, 1)
        .reshape(R, NPP)
        .astype(np.float16)
    )
    rbf_t[R] = 1.0  # ones row: contracts with the b1 row of w1a

    flat = (
        nb_c.reshape(GROUPS, 128, NB).transpose(0, 2, 1).reshape(NPP).astype(np.int16)
    )
    # dma_gather index layout: element i at [i % 16, i // 16], replicated x8
    idx16 = np.ascontiguousarray(flat.reshape(NPP // 16, 16).T)
    idx = np.ascontiguousarray(np.tile(idx16, (8, 1)))

    m = {
        "x": x16,
        "rbf_t": rbf_t,
        "idx": idx,
        "w1": w1a_16,
        "w2": w2_16,
    }
    if b2rep is not None:
        m["b2rep"] = b2rep
    return m


def kernel(x, rbf, neighbors, w1, b1, w2, b2):
    from concourse.bass_utils import run_bass_kernel_spmd

    x = np.asarray(x)
    rbf = np.asarray(rbf)
    neighbors = np.asarray(neighbors)
    w1 = np.asarray(w1)
    b1 = np.asarray(b1)
    w2 = np.asarray(w2)
    b2 = np.asarray(b2)

    with_b2 = bool(np.any(b2 != 0))
    nc = _build(with_b2)

    x16 = x.astype(np.float16)
    w1a_16 = np.ascontiguousarray(
        np.vstack([w1, b1.reshape(1, F)]).astype(np.float16)
    )
    w2_16 = np.ascontiguousarray(w2.astype(np.float16))
    b2rep = (
        np.ascontiguousarray(np.tile(b2.astype(np.float16), 8).reshape(1, 1024))
        if with_b2
        else None
    )

    in_maps = [
        _prep_core_inputs(x16, rbf, neighbors, w1a_16, w2_16, b2rep, c)
        for c in range(NCORES)
    ]

    # Transient NRT_EXEC_UNIT_UNRECOVERABLE wedges clear on re-execution;
    # retry a couple of times before giving up.
    last_exc = None
    for attempt in range(3):
        try:
            res = run_bass_kernel_spmd(
                nc,
                in_maps,
                core_ids=list(range(NCORES)),
                trace=bool(int(os.environ.get("CFCONV_TRACE", "0"))),
            )
            break
        except Exception as e:  # noqa: BLE001
            last_exc = e
            import time

            time.sleep(2.0)
    else:
        raise last_exc
    _CACHE["last_result"] = res

    out = np.concatenate([res.results[c]["out"][:NA] for c in range(NCORES)], axis=0)
    return np.ascontiguousarray(out.astype(np.float32))

